# revision 26
# baseline (speedup 1.0000x reference)
"""Trainium2 Bass kernel v5 for nn_BaseAttention (B=4, H=16, S=2048, D=64, key-mask).

Strategy (8 NeuronCores, 8 heads/core, all heads of a core share one batch's
mask):
  Host-side prep (exactness-preserving):
    - Key compaction: only unmasked keys are shipped (the reference's -10000
      additive mask zeroes masked keys exactly in fp32); K,V rows gathered by
      mask==0, zero-padded to S_K = T_K*128.
    - Q^T/K^T transposed on host and duplicated onto partitions 64..127
      (bf16) so mm1 runs two 64-row matmuls concurrently in the two PE row
      halves. V is swizzled to [128, T_K, 64] (no ones column — the softmax
      denominator has its own matmuls now).
  Device per macro (head, window-quad wq of 4x256 queries, k-tile j):
    - mm1: 4 matmuls (4 windows, row-tiled concurrent pairs) write S^T fp32
      into one PSUM tile [128, 4, 256] (2 banks; TRN2 matmul cannot emit
      16-bit PSUM).
    - exp: ONE ScalarE activation over the whole macro tile (FD=4*WSC,
      amortizing the ~260ns fixed ACT cost) for the first WSC columns of
      each window; the DVE covers the last DC=256-WSC columns with a
      two-point Schraudolph (one-point measured 2.2e-2 end-to-end: too big).
    - mm2: V-stationary, col-tiled: windows (2r, 2r+1) run CONCURRENTLY in
      PE column groups 0-1 / 2-3 (M=64 each), accumulating out^T into a
      single PSUM bank [128, 2, 256] (4 windows packed by partition-half x
      column-half).
    - den: 4 concurrent col-tiled M=1 matmuls (masked-ones weights kill the
      zero-padded keys) accumulate the 4 windows' softmax denominators into
      rows 0/32/64/96 of one PSUM bank.
    - epilogue: DVE drains acc+den to SBUF, DMA raw accumulators out; host
      divides and transposes.
  PSUM budget: st 2 banks x2 bufs + acc 1 bank x2 + den 1 bank x2 = 8.
  Emission is a flat software pipeline over macros with mm2/den and drains
  lagging so the in-order engine queues never head-block.

Self-contained: hardcodes shapes; imports concourse from /opt/trn_rl_repo.
"""

import sys

if "/opt/trn_rl_repo" not in sys.path:
    sys.path.insert(0, "/opt/trn_rl_repo")

import numpy as np
import ml_dtypes

import concourse.bass as bass
import concourse.mybir as mybir
import concourse.tile as tile
from concourse import bacc

F32 = mybir.dt.float32
BF16 = mybir.dt.bfloat16
I16 = mybir.dt.int16

N_CORES = 8
B, NH, S, D = 4, 16, 2048, 64
H = (B * NH) // N_CORES  # heads per core = 8
P = 128
W = 256                  # query window
NWIN = S // W            # 8 windows per head
NWQ = NWIN // 4          # 2 window-quads per head
SCALE = 1.0 / 8.0

# Two-point Schraudolph exp on DVE: exp(x) ~ S(x+h) + S(x-h) where S is the
# classic bitcast-exp (tensor_scalar ->int16 whose int16 bits are the bf16
# pattern of exp/2cosh(h)); averaging two quarter-period-shifted sawtooths
# cuts the interpolation ripple to ~0.8% rms. The 1/(2cosh h) factor is
# folded into the bias constants.
LOG2E = 1.4426950408889634
INT_SHIFT = 78.0
_H_SH = INT_SHIFT / (256.0 * LOG2E)  # ~0.2112
_BS = 16256.0 - 128.0 * float(np.log2(2.0 * np.cosh(_H_SH))) - 7.0
EXP_A = 128.0 * LOG2E * SCALE
EXP_B_P = _BS + 128.0 * LOG2E * _H_SH
# macro i -> DVE iff i % ASSIGN_MOD in DVE_SLOTS (~2/7 of key-tiles approx);
# whole-macro grains amortize each engine's fixed per-instruction cost over
# FD=1024 (a column-split pays DVE fixed costs on ~250-element slices, which
# measured 3.7 ns/elem vs ScalarE's 1.17). First/last macros stay on ScalarE
# so the pipeline head/tail never waits on a long (~2.3us) DVE macro.
ASSIGN_MOD = 7
DVE_SLOTS = (2, 5)
MM2_LAG = 3
DRAIN_LAG = 5


def emit_core_program(ctx, nc, tc, qT_h, kT_h, vP_h, mk_h, accO_h, denO_h, T_K):
    """qT: [H,128,S] bf16; kT: [H,128,S_K] bf16; vP: [H,128,T_K,64] bf16;
    mk: [128,T_K,32] bf16 key-validity (col 0; cols 1-31 zero so den
    matmuls write all partitions, keeping the drained tile fully init); accO: [H,2,128,2,256] f32 raw numerator
    accumulators; denO: [H,2,128,256] f32 (rows 0/32/64/96 are the 4 windows'
    denominators). Host divides/transposes."""
    pool = lambda *a, **kw: ctx.enter_context(tc.tile_pool(*a, **kw))
    singles = pool(name="singles", bufs=1)
    ld = pool(name="ld", bufs=2)
    pT_pool = pool(name="pT", bufs=4)
    pab_pool = pool(name="pab", bufs=2)
    sbst_pool = pool(name="sbst", bufs=2)
    st_pool = pool(name="st", bufs=2, space="PSUM")    # [128,4,256] f32 = 2 banks
    acc_pool = pool(name="acc", bufs=2, space="PSUM")  # [128,2,256] f32 = 1 bank
    den_pool = pool(name="den", bufs=2, space="PSUM")  # [128,256] f32 = 1 bank

    # Pull the ~2.7us exp table load under the first input DMAs.
    scr = singles.tile([P, 8], F32, name="scr")
    nc.vector.memset(scr, 0.0)
    nc.scalar.activation(
        out=scr, in_=scr, func=mybir.ActivationFunctionType.Exp, scale=1.0
    )
    mk_sb = singles.tile([P, T_K, 32], BF16, name="mk_sb")

    def emit_head_load(h):
        q_sb = ld.tile([P, S], BF16, tag="q_sb", name=f"q_sb_{h}")
        k_sb = ld.tile([P, T_K * P], BF16, tag="k_sb", name=f"k_sb_{h}")
        v_sb = ld.tile([P, T_K, D], BF16, tag="v_sb", name=f"v_sb_{h}")
        if h == 0:
            # warmup: split first loads so macro 0 (k-tiles 0-1, windows 0-3)
            # can start ASAP; spread issue over two HWDGE queues (the
            # Activation queue is idle before the first exp)
            nc.scalar.dma_start(out=k_sb, in_=kT_h[h])
            nc.sync.dma_start(out=q_sb[:, 0 : 4 * W], in_=qT_h[h][:, 0 : 4 * W])
            nc.sync.dma_start(out=q_sb[:, 4 * W :], in_=qT_h[h][:, 4 * W :])
            nc.scalar.dma_start(out=v_sb, in_=vP_h[h])
            nc.scalar.dma_start(out=mk_sb, in_=mk_h)
        else:
            nc.sync.dma_start(out=q_sb, in_=qT_h[h])
            nc.sync.dma_start(out=k_sb, in_=kT_h[h])
            nc.sync.dma_start(out=v_sb, in_=vP_h[h])
        return q_sb, k_sb, v_sb

    # macros: (h, wq, j) — one k-tile of one window-quad
    macros = [
        (h, wq, j)
        for h in range(H)
        for wq in range(NWQ)
        for j in range(T_K)
    ]
    heads = {0: emit_head_load(0)}
    accden = {}
    pTs = {}
    pending_drain = []

    def emit_front(i):
        h, wq, j = macros[i]
        if wq == 0 and j == 0 and h > 1:
            del heads[h - 2]
        q_sb, k_sb, _ = heads[h]
        if j == 0:
            acc = acc_pool.tile([P, 2, W], F32, tag="acc", name=f"acc_{h}_{wq}")
            den = den_pool.tile([P, W], F32, tag="den", name=f"den_{h}_{wq}")
            accden[(h, wq)] = (acc, den)
        st = st_pool.tile([P, 4, W], F32, tag="st", name=f"st_{i}")
        # Concurrent row-group matmuls draining into the SAME psum bank are a
        # fatal HW collision: map window ww -> slot so row-group-0 windows
        # (ww even) fill bank 0 (slots 0,1) and row-group-1 windows fill bank
        # 1 (slots 2,3), while emission alternates groups for concurrency.
        for ww in range(4):
            c = ww & 1
            lo = D * c
            slot = 2 * c + (ww >> 1)
            wcol = (wq * 4 + ww) * W
            nc.tensor.matmul(
                st[:, slot, :],
                lhsT=k_sb[lo : lo + D, j * P : (j + 1) * P],
                rhs=q_sb[lo : lo + D, wcol : wcol + W],
                start=True,
                stop=True,
            )
        pT = pT_pool.tile([P, 4, W], BF16, tag="pT", name=f"pT_{i}")
        if i % ASSIGN_MOD in DVE_SLOTS and 2 <= i < len(macros) - 4:
            pa = pab_pool.tile([P, 4, W], I16, tag="pa", name=f"pa_{i}")
            pb = pab_pool.tile([P, 4, W], I16, tag="pb", name=f"pb_{i}")
            nc.vector.tensor_scalar(
                pa, st, EXP_A, EXP_B_P,
                mybir.AluOpType.mult, mybir.AluOpType.add,
            )
            nc.vector.tensor_scalar(
                pb, pa, -INT_SHIFT, 0.0,
                mybir.AluOpType.add, mybir.AluOpType.add,
            )
            nc.vector.tensor_add(pT, pa.bitcast(BF16), pb.bitcast(BF16))
        else:
            nc.scalar.activation(
                out=pT,
                in_=st,
                func=mybir.ActivationFunctionType.Exp,
                scale=SCALE,
            )
        pTs[i] = pT
        # prefetch the next head early in this head's macro stream
        if i % (NWQ * T_K) == min(1, NWQ * T_K - 1) and h + 1 < H:
            heads[h + 1] = emit_head_load(h + 1)

    def emit_back(i):
        h, wq, j = macros[i]
        v_sb = heads[h][2]
        pT = pTs.pop(i)
        acc, den = accden[(h, wq)]
        first = j == 0
        last = j == T_K - 1
        # acc packs 4 windows into ONE psum bank (2 partition-halves x 2
        # column-halves); den packs 4 M=1 rows into one bank. The has_written
        # clear on start=True covers the zero_region (the 2KB bank) of the
        # WRITTEN partitions only, so each partition-half carries start on
        # its first matmul (r==0) — the r==1 window's bytes land on cleared
        # bits and overwrite — and stop on its last. The sim's zero-region
        # group tracker aliases these sub-bank groups -> skip_group_check.
        slot = lambda ww: 2 * (ww & 1) + (ww >> 1)
        for r in range(2):
            nc.tensor.matmul(
                acc[0:D, r, :],
                lhsT=v_sb[:, j, :],
                rhs=pT[:, slot(2 * r), :],
                start=first and r == 0,
                stop=last and r == 1,
                skip_group_check=True,
            )
            nc.tensor.matmul(
                acc[D : 2 * D, r, :],
                lhsT=v_sb[:, j, :],
                rhs=pT[:, slot(2 * r + 1), :],
                start=first and r == 0,
                stop=last and r == 1,
                skip_group_check=True,
            )
        for g in range(4):
            nc.tensor.matmul(
                den[32 * g : 32 * g + 32, :],
                lhsT=mk_sb[:, j, :],
                rhs=pT[:, slot(g), :],
                start=first,
                stop=last,
                tile_position=(0, 32 * g),
                skip_group_check=True,
            )
        if last:
            del accden[(h, wq)]
            pending_drain.append((i + DRAIN_LAG, (h, wq, acc, den)))

    def emit_drain(ep):
        h, wq, acc, den = ep
        accs = sbst_pool.tile([P, 2, W], F32, tag="accs", name=f"accs_{h}_{wq}")
        nc.vector.tensor_copy(accs, acc)
        dens = sbst_pool.tile([P, W], F32, tag="dens", name=f"dens_{h}_{wq}")
        nc.vector.tensor_copy(dens, den)
        nc.sync.dma_start(out=accO_h[h][wq], in_=accs)
        nc.sync.dma_start(out=denO_h[h][wq], in_=dens)

    n = len(macros)
    for i in range(n):
        emit_front(i)
        if i >= MM2_LAG:
            emit_back(i - MM2_LAG)
        while pending_drain and pending_drain[0][0] <= i - MM2_LAG:
            emit_drain(pending_drain.pop(0)[1])
    for i in range(n - MM2_LAG, n):
        emit_back(i)
        while pending_drain and pending_drain[0][0] <= i:
            emit_drain(pending_drain.pop(0)[1])
    for _, ep in pending_drain:
        emit_drain(ep)


def build_nc(T_K):
    S_K = T_K * P
    nc = bacc.Bacc("TRN2", target_bir_lowering=False, debug=False, num_devices=N_CORES)
    qT = nc.declare_dram_parameter("qT", [H, P, S], BF16, isOutput=False)
    kT = nc.declare_dram_parameter("kT", [H, P, S_K], BF16, isOutput=False)
    vP = nc.declare_dram_parameter("vP", [H, P, T_K, D], BF16, isOutput=False)
    mk = nc.declare_dram_parameter("mk", [P, T_K, 32], BF16, isOutput=False)
    accO = nc.declare_dram_parameter("acc", [H, NWQ, P, 2, W], F32, isOutput=True)
    denO = nc.declare_dram_parameter("den", [H, NWQ, P, W], F32, isOutput=True)
    from contextlib import ExitStack

    with tile.TileContext(nc) as tc, ExitStack() as ctx:
        emit_core_program(
            ctx, nc, tc, qT.ap(), kT.ap(), vP.ap(), mk.ap(), accO.ap(), denO.ap(), T_K
        )
    nc.compile()
    return nc


_NC_CACHE = {}


def get_nc(T_K):
    if T_K not in _NC_CACHE:
        _NC_CACHE[T_K] = build_nc(T_K)
    return _NC_CACHE[T_K]


def make_in_maps(q, k, v, mask):
    """Host prep: compaction, transposes, duplication, swizzle, bf16 cast."""
    bf16 = ml_dtypes.bfloat16
    qf = np.asarray(q, dtype=np.float32).reshape(B * NH, S, D)
    kf = np.asarray(k, dtype=np.float32).reshape(B * NH, S, D)
    vf = np.asarray(v, dtype=np.float32).reshape(B * NH, S, D)
    mf = np.asarray(mask, dtype=np.int32).reshape(B, S)
    idxs = [np.nonzero(mf[b] == 0)[0] for b in range(B)]
    max_nu = max(len(ix) for ix in idxs)
    T_K = max(1, -(-max_nu // P))  # ceil
    S_K = T_K * P

    in_maps = []
    for c in range(N_CORES):
        lo = c * H
        b = lo // NH
        ix = idxs[b]
        nu = len(ix)
        qT = np.empty((H, P, S), dtype=bf16)
        kT = np.zeros((H, P, S_K), dtype=bf16)
        vP = np.zeros((H, P, T_K, D), dtype=bf16)
        mk = np.zeros((P, T_K, 32), dtype=bf16)
        kidx = np.arange(T_K)[None, :] * P + np.arange(P)[:, None]
        mk[:, :, 0] = (kidx < nu).astype(bf16)  # valid iff t*128+p < nu
        for hh in range(H):
            qt = np.ascontiguousarray(qf[lo + hh].T).astype(bf16)  # [D, S]
            qT[hh, 0:D] = qt
            qT[hh, D : 2 * D] = qt
            kg = kf[lo + hh][ix]  # [nu, D]
            kt = kg.T.astype(bf16)  # [D, nu]
            kT[hh, 0:D, 0:nu] = kt
            kT[hh, D : 2 * D, 0:nu] = kt
            vg = np.zeros((S_K, D), dtype=np.float32)
            vg[0:nu] = vf[lo + hh][ix]
            vP[hh] = vg.reshape(T_K, P, D).transpose(1, 0, 2).astype(bf16)
        in_maps.append({"qT": qT, "kT": kT, "vP": vP, "mk": mk})
    return in_maps, T_K


def kernel(q, k, v, mask):
    from concourse.bass_utils import run_bass_kernel_spmd

    in_maps, T_K = make_in_maps(q, k, v, mask)
    nc = get_nc(T_K)
    try:
        res = run_bass_kernel_spmd(nc, in_maps, list(range(N_CORES))).results
    except Exception:
        # transient INTERNAL error after a fresh NEFF compile; retry clears it
        res = run_bass_kernel_spmd(nc, in_maps, list(range(N_CORES))).results
    out = np.empty((B * NH, S, D), dtype=np.float32)
    for c in range(N_CORES):
        acc = res[c]["acc"]  # [H, NWQ, 128, 2, 256]
        den = res[c]["den"]  # [H, NWQ, 128, 256]
        for hh in range(H):
            for wq in range(NWQ):
                a = acc[hh, wq]
                d = den[hh, wq]
                for ww in range(4):
                    w = wq * 4 + ww
                    nums = a[(ww & 1) * D : (ww & 1) * D + D, ww >> 1, :]  # [64,256]
                    dd = d[32 * ww, :]  # [256]
                    out[c * H + hh, w * W : (w + 1) * W, :] = (nums / dd).T
    return out.reshape(B, NH, S, D)


if __name__ == "__main__":
    nc = build_nc(9)
    print("built ok")


# revision 27
# speedup vs baseline: 1.1775x; 1.1775x over previous
"""Trainium2 Bass kernel v5 for nn_BaseAttention (B=4, H=16, S=2048, D=64, key-mask).

Strategy (8 NeuronCores, 8 heads/core, all heads of a core share one batch's
mask):
  Host-side prep (exactness-preserving):
    - Key compaction: only unmasked keys are shipped (the reference's -10000
      additive mask zeroes masked keys exactly in fp32); K,V rows gathered by
      mask==0, zero-padded to S_K = T_K*128.
    - Q^T/K^T transposed on host and duplicated onto partitions 64..127
      (bf16) so mm1 runs two 64-row matmuls concurrently in the two PE row
      halves. V is swizzled to [128, T_K, 64] (no ones column — the softmax
      denominator has its own matmuls now).
  Device per macro (head, window-quad wq of 4x256 queries, k-tile j):
    - mm1: 4 matmuls (4 windows, row-tiled concurrent pairs) write S^T fp32
      into one PSUM tile [128, 4, 256] (2 banks; TRN2 matmul cannot emit
      16-bit PSUM).
    - exp: ONE ScalarE activation over the whole macro tile (FD=4*WSC,
      amortizing the ~260ns fixed ACT cost) for the first WSC columns of
      each window; the DVE covers the last DC=256-WSC columns with a
      two-point Schraudolph (one-point measured 2.2e-2 end-to-end: too big).
    - mm2: V-stationary, col-tiled: windows (2r, 2r+1) run CONCURRENTLY in
      PE column groups 0-1 / 2-3 (M=64 each), accumulating out^T into a
      single PSUM bank [128, 2, 256] (4 windows packed by partition-half x
      column-half).
    - den: 4 concurrent col-tiled M=1 matmuls (masked-ones weights kill the
      zero-padded keys) accumulate the 4 windows' softmax denominators into
      rows 0/32/64/96 of one PSUM bank.
    - epilogue: DVE drains acc+den to SBUF, DMA raw accumulators out; host
      divides and transposes.
  PSUM budget: st 2 banks x2 bufs + acc 1 bank x2 + den 1 bank x2 = 8.
  Emission is a flat software pipeline over macros with mm2/den and drains
  lagging so the in-order engine queues never head-block.

Self-contained: hardcodes shapes; imports concourse from /opt/trn_rl_repo.
"""

import sys

if "/opt/trn_rl_repo" not in sys.path:
    sys.path.insert(0, "/opt/trn_rl_repo")

import numpy as np
import ml_dtypes

import concourse.bass as bass
import concourse.mybir as mybir
import concourse.tile as tile
from concourse import bacc

F32 = mybir.dt.float32
BF16 = mybir.dt.bfloat16
I16 = mybir.dt.int16

N_CORES = 8
B, NH, S, D = 4, 16, 2048, 64
H = (B * NH) // N_CORES  # heads per core = 8
P = 128
W = 256                  # query window
NWIN = S // W            # 8 windows per head
NWQ = NWIN // 4          # 2 window-quads per head
SCALE = 1.0 / 8.0

# Two-point Schraudolph exp on DVE: exp(x) ~ S(x+h) + S(x-h) where S is the
# classic bitcast-exp (tensor_scalar ->int16 whose int16 bits are the bf16
# pattern of exp/2cosh(h)); averaging two quarter-period-shifted sawtooths
# cuts the interpolation ripple to ~0.8% rms. The 1/(2cosh h) factor is
# folded into the bias constants.
LOG2E = 1.4426950408889634
INT_SHIFT = 78.0
_H_SH = INT_SHIFT / (256.0 * LOG2E)  # ~0.2112
_BS = 16256.0 - 128.0 * float(np.log2(2.0 * np.cosh(_H_SH))) - 7.0
EXP_A = 128.0 * LOG2E * SCALE
EXP_B_P = _BS + 128.0 * LOG2E * _H_SH
# macro i -> DVE iff i % ASSIGN_MOD in DVE_SLOTS (~2/7 of key-tiles approx);
# whole-macro grains amortize each engine's fixed per-instruction cost over
# FD=1024 (a column-split pays DVE fixed costs on ~250-element slices, which
# measured 3.7 ns/elem vs ScalarE's 1.17). First/last macros stay on ScalarE
# so the pipeline head/tail never waits on a long (~2.3us) DVE macro.
ASSIGN_MOD = 7
DVE_SLOTS = (2, 5)
MM2_LAG = 3
DRAIN_LAG = 5


def emit_core_program(ctx, nc, tc, qT_h, kT_h, vP_h, mk_h, accO_h, denO_h, T_K):
    """qT: [H,128,S] bf16; kT: [H,128,S_K] bf16; vP: [H,128,T_K,64] bf16;
    mk: [128,T_K,32] bf16 key-validity (col 0; cols 1-31 zero so den
    matmuls write all partitions, keeping the drained tile fully init); accO: [H,2,128,2,256] f32 raw numerator
    accumulators; denO: [H,2,128,256] f32 (rows 0/32/64/96 are the 4 windows'
    denominators). Host divides/transposes."""
    pool = lambda *a, **kw: ctx.enter_context(tc.tile_pool(*a, **kw))
    singles = pool(name="singles", bufs=1)
    ld = pool(name="ld", bufs=2)
    pT_pool = pool(name="pT", bufs=4)
    pab_pool = pool(name="pab", bufs=2)
    sbst_pool = pool(name="sbst", bufs=2)
    st_pool = pool(name="st", bufs=2, space="PSUM")    # [128,4,256] f32 = 2 banks
    acc_pool = pool(name="acc", bufs=2, space="PSUM")  # [128,2,256] f32 = 1 bank
    den_pool = pool(name="den", bufs=2, space="PSUM")  # [128,256] f32 = 1 bank

    # Pull the ~2.7us exp table load under the first input DMAs.
    scr = singles.tile([P, 8], F32, name="scr")
    nc.vector.memset(scr, 0.0)
    nc.scalar.activation(
        out=scr, in_=scr, func=mybir.ActivationFunctionType.Exp, scale=1.0
    )
    mk_sb = singles.tile([P, T_K, 32], BF16, name="mk_sb")

    def emit_head_load(h):
        q_sb = ld.tile([P, S], BF16, tag="q_sb", name=f"q_sb_{h}")
        k_sb = ld.tile([P, T_K * P], BF16, tag="k_sb", name=f"k_sb_{h}")
        v_sb = ld.tile([P, T_K, D], BF16, tag="v_sb", name=f"v_sb_{h}")
        if h == 0:
            # warmup: split first loads so macro 0 (k-tiles 0-1, windows 0-3)
            # can start ASAP; spread issue over two HWDGE queues (the
            # Activation queue is idle before the first exp)
            nc.scalar.dma_start(out=k_sb[:, 0:P], in_=kT_h[h][:, 0:P])
            nc.sync.dma_start(out=q_sb[:, 0 : 4 * W], in_=qT_h[h][:, 0 : 4 * W])
            if T_K > 1:
                nc.scalar.dma_start(out=k_sb[:, P:], in_=kT_h[h][:, P:])
            nc.sync.dma_start(out=q_sb[:, 4 * W :], in_=qT_h[h][:, 4 * W :])
            nc.scalar.dma_start(out=v_sb, in_=vP_h[h])
            nc.scalar.dma_start(out=mk_sb, in_=mk_h)
        else:
            nc.sync.dma_start(out=q_sb, in_=qT_h[h])
            nc.sync.dma_start(out=k_sb, in_=kT_h[h])
            nc.sync.dma_start(out=v_sb, in_=vP_h[h])
        return q_sb, k_sb, v_sb

    # macros: (h, wq, j) — one k-tile of one window-quad
    macros = [
        (h, wq, j)
        for h in range(H)
        for wq in range(NWQ)
        for j in range(T_K)
    ]
    heads = {0: emit_head_load(0)}
    accden = {}
    pTs = {}
    pending_drain = []

    def emit_front(i):
        h, wq, j = macros[i]
        if wq == 0 and j == 0 and h > 1:
            del heads[h - 2]
        q_sb, k_sb, _ = heads[h]
        if j == 0:
            acc = acc_pool.tile([P, 2, W], F32, tag="acc", name=f"acc_{h}_{wq}")
            den = den_pool.tile([P, W], F32, tag="den", name=f"den_{h}_{wq}")
            accden[(h, wq)] = (acc, den)
        st = st_pool.tile([P, 4, W], F32, tag="st", name=f"st_{i}")
        # Concurrent row-group matmuls draining into the SAME psum bank are a
        # fatal HW collision: map window ww -> slot so row-group-0 windows
        # (ww even) fill bank 0 (slots 0,1) and row-group-1 windows fill bank
        # 1 (slots 2,3), while emission alternates groups for concurrency.
        for ww in range(4):
            c = ww & 1
            lo = D * c
            slot = 2 * c + (ww >> 1)
            wcol = (wq * 4 + ww) * W
            nc.tensor.matmul(
                st[:, slot, :],
                lhsT=k_sb[lo : lo + D, j * P : (j + 1) * P],
                rhs=q_sb[lo : lo + D, wcol : wcol + W],
                start=True,
                stop=True,
            )
        pT = pT_pool.tile([P, 4, W], BF16, tag="pT", name=f"pT_{i}")
        if i % ASSIGN_MOD in DVE_SLOTS and 2 <= i < len(macros) - 4:
            pa = pab_pool.tile([P, 4, W], I16, tag="pa", name=f"pa_{i}")
            pb = pab_pool.tile([P, 4, W], I16, tag="pb", name=f"pb_{i}")
            nc.vector.tensor_scalar(
                pa, st, EXP_A, EXP_B_P,
                mybir.AluOpType.mult, mybir.AluOpType.add,
            )
            nc.vector.tensor_scalar(
                pb, pa, -INT_SHIFT, 0.0,
                mybir.AluOpType.add, mybir.AluOpType.add,
            )
            nc.vector.tensor_add(pT, pa.bitcast(BF16), pb.bitcast(BF16))
        else:
            nc.scalar.activation(
                out=pT,
                in_=st,
                func=mybir.ActivationFunctionType.Exp,
                scale=SCALE,
            )
        pTs[i] = pT
        # prefetch the next head early in this head's macro stream
        if i % (NWQ * T_K) == min(1, NWQ * T_K - 1) and h + 1 < H:
            heads[h + 1] = emit_head_load(h + 1)

    def emit_back(i):
        h, wq, j = macros[i]
        v_sb = heads[h][2]
        pT = pTs.pop(i)
        acc, den = accden[(h, wq)]
        first = j == 0
        last = j == T_K - 1
        # acc packs 4 windows into ONE psum bank (2 partition-halves x 2
        # column-halves); den packs 4 M=1 rows into one bank. The has_written
        # clear on start=True covers the zero_region (the 2KB bank) of the
        # WRITTEN partitions only, so each partition-half carries start on
        # its first matmul (r==0) — the r==1 window's bytes land on cleared
        # bits and overwrite — and stop on its last. The sim's zero-region
        # group tracker aliases these sub-bank groups -> skip_group_check.
        slot = lambda ww: 2 * (ww & 1) + (ww >> 1)
        for r in range(2):
            nc.tensor.matmul(
                acc[0:D, r, :],
                lhsT=v_sb[:, j, :],
                rhs=pT[:, slot(2 * r), :],
                start=first and r == 0,
                stop=last and r == 1,
                skip_group_check=True,
            )
            nc.tensor.matmul(
                acc[D : 2 * D, r, :],
                lhsT=v_sb[:, j, :],
                rhs=pT[:, slot(2 * r + 1), :],
                start=first and r == 0,
                stop=last and r == 1,
                skip_group_check=True,
            )
        for g in range(4):
            nc.tensor.matmul(
                den[32 * g : 32 * g + 32, :],
                lhsT=mk_sb[:, j, :],
                rhs=pT[:, slot(g), :],
                start=first,
                stop=last,
                tile_position=(0, 32 * g),
                skip_group_check=True,
            )
        if last:
            del accden[(h, wq)]
            pending_drain.append((i + DRAIN_LAG, (h, wq, acc, den)))

    def emit_drain(ep):
        h, wq, acc, den = ep
        accs = sbst_pool.tile([P, 2, W], F32, tag="accs", name=f"accs_{h}_{wq}")
        nc.vector.tensor_copy(accs, acc)
        dens = sbst_pool.tile([P, W], F32, tag="dens", name=f"dens_{h}_{wq}")
        nc.vector.tensor_copy(dens, den)
        nc.sync.dma_start(out=accO_h[h][wq], in_=accs)
        nc.sync.dma_start(out=denO_h[h][wq], in_=dens)

    n = len(macros)
    for i in range(n):
        emit_front(i)
        if i >= MM2_LAG:
            emit_back(i - MM2_LAG)
        while pending_drain and pending_drain[0][0] <= i - MM2_LAG:
            emit_drain(pending_drain.pop(0)[1])
    for i in range(n - MM2_LAG, n):
        emit_back(i)
        while pending_drain and pending_drain[0][0] <= i:
            emit_drain(pending_drain.pop(0)[1])
    for _, ep in pending_drain:
        emit_drain(ep)


def build_nc(T_K):
    S_K = T_K * P
    nc = bacc.Bacc("TRN2", target_bir_lowering=False, debug=False, num_devices=N_CORES)
    qT = nc.declare_dram_parameter("qT", [H, P, S], BF16, isOutput=False)
    kT = nc.declare_dram_parameter("kT", [H, P, S_K], BF16, isOutput=False)
    vP = nc.declare_dram_parameter("vP", [H, P, T_K, D], BF16, isOutput=False)
    mk = nc.declare_dram_parameter("mk", [P, T_K, 32], BF16, isOutput=False)
    accO = nc.declare_dram_parameter("acc", [H, NWQ, P, 2, W], F32, isOutput=True)
    denO = nc.declare_dram_parameter("den", [H, NWQ, P, W], F32, isOutput=True)
    from contextlib import ExitStack

    with tile.TileContext(nc) as tc, ExitStack() as ctx:
        emit_core_program(
            ctx, nc, tc, qT.ap(), kT.ap(), vP.ap(), mk.ap(), accO.ap(), denO.ap(), T_K
        )
    nc.compile()
    return nc


_NC_CACHE = {}


def get_nc(T_K):
    if T_K not in _NC_CACHE:
        _NC_CACHE[T_K] = build_nc(T_K)
    return _NC_CACHE[T_K]


def make_in_maps(q, k, v, mask):
    """Host prep: compaction, transposes, duplication, swizzle, bf16 cast."""
    bf16 = ml_dtypes.bfloat16
    qf = np.asarray(q, dtype=np.float32).reshape(B * NH, S, D)
    kf = np.asarray(k, dtype=np.float32).reshape(B * NH, S, D)
    vf = np.asarray(v, dtype=np.float32).reshape(B * NH, S, D)
    mf = np.asarray(mask, dtype=np.int32).reshape(B, S)
    idxs = [np.nonzero(mf[b] == 0)[0] for b in range(B)]
    max_nu = max(len(ix) for ix in idxs)
    T_K = max(1, -(-max_nu // P))  # ceil
    S_K = T_K * P

    in_maps = []
    for c in range(N_CORES):
        lo = c * H
        b = lo // NH
        ix = idxs[b]
        nu = len(ix)
        qT = np.empty((H, P, S), dtype=bf16)
        kT = np.zeros((H, P, S_K), dtype=bf16)
        vP = np.zeros((H, P, T_K, D), dtype=bf16)
        mk = np.zeros((P, T_K, 32), dtype=bf16)
        kidx = np.arange(T_K)[None, :] * P + np.arange(P)[:, None]
        mk[:, :, 0] = (kidx < nu).astype(bf16)  # valid iff t*128+p < nu
        for hh in range(H):
            qt = np.ascontiguousarray(qf[lo + hh].T).astype(bf16)  # [D, S]
            qT[hh, 0:D] = qt
            qT[hh, D : 2 * D] = qt
            kg = kf[lo + hh][ix]  # [nu, D]
            kt = kg.T.astype(bf16)  # [D, nu]
            kT[hh, 0:D, 0:nu] = kt
            kT[hh, D : 2 * D, 0:nu] = kt
            vg = np.zeros((S_K, D), dtype=np.float32)
            vg[0:nu] = vf[lo + hh][ix]
            vP[hh] = vg.reshape(T_K, P, D).transpose(1, 0, 2).astype(bf16)
        in_maps.append({"qT": qT, "kT": kT, "vP": vP, "mk": mk})
    return in_maps, T_K


def kernel(q, k, v, mask):
    from concourse.bass_utils import run_bass_kernel_spmd

    in_maps, T_K = make_in_maps(q, k, v, mask)
    nc = get_nc(T_K)
    try:
        res = run_bass_kernel_spmd(nc, in_maps, list(range(N_CORES))).results
    except Exception:
        # transient INTERNAL error after a fresh NEFF compile; retry clears it
        res = run_bass_kernel_spmd(nc, in_maps, list(range(N_CORES))).results
    out = np.empty((B * NH, S, D), dtype=np.float32)
    for c in range(N_CORES):
        acc = res[c]["acc"]  # [H, NWQ, 128, 2, 256]
        den = res[c]["den"]  # [H, NWQ, 128, 256]
        for hh in range(H):
            for wq in range(NWQ):
                a = acc[hh, wq]
                d = den[hh, wq]
                for ww in range(4):
                    w = wq * 4 + ww
                    nums = a[(ww & 1) * D : (ww & 1) * D + D, ww >> 1, :]  # [64,256]
                    dd = d[32 * ww, :]  # [256]
                    out[c * H + hh, w * W : (w + 1) * W, :] = (nums / dd).T
    return out.reshape(B, NH, S, D)


if __name__ == "__main__":
    nc = build_nc(9)
    print("built ok")


# revision 28
# speedup vs baseline: 1.1925x; 1.0128x over previous
"""Trainium2 Bass kernel v5 for nn_BaseAttention (B=4, H=16, S=2048, D=64, key-mask).

Strategy (8 NeuronCores, 8 heads/core, all heads of a core share one batch's
mask):
  Host-side prep (exactness-preserving):
    - Key compaction: only unmasked keys are shipped (the reference's -10000
      additive mask zeroes masked keys exactly in fp32); K,V rows gathered by
      mask==0, zero-padded to S_K = T_K*128.
    - Q^T/K^T transposed on host and duplicated onto partitions 64..127
      (bf16) so mm1 runs two 64-row matmuls concurrently in the two PE row
      halves. V is swizzled to [128, T_K, 64] (no ones column — the softmax
      denominator has its own matmuls now).
  Device per macro (head, window-quad wq of 4x256 queries, k-tile j):
    - mm1: 4 matmuls (4 windows, row-tiled concurrent pairs) write S^T fp32
      into one PSUM tile [128, 4, 256] (2 banks; TRN2 matmul cannot emit
      16-bit PSUM).
    - exp: ONE ScalarE activation over the whole macro tile (FD=4*WSC,
      amortizing the ~260ns fixed ACT cost) for the first WSC columns of
      each window; the DVE covers the last DC=256-WSC columns with a
      two-point Schraudolph (one-point measured 2.2e-2 end-to-end: too big).
    - mm2: V-stationary, col-tiled: windows (2r, 2r+1) run CONCURRENTLY in
      PE column groups 0-1 / 2-3 (M=64 each), accumulating out^T into a
      single PSUM bank [128, 2, 256] (4 windows packed by partition-half x
      column-half).
    - den: 4 concurrent col-tiled M=1 matmuls (masked-ones weights kill the
      zero-padded keys) accumulate the 4 windows' softmax denominators into
      rows 0/32/64/96 of one PSUM bank.
    - epilogue: DVE drains acc+den to SBUF, DMA raw accumulators out; host
      divides and transposes.
  PSUM budget: st 2 banks x2 bufs + acc 1 bank x2 + den 1 bank x2 = 8.
  Emission is a flat software pipeline over macros with mm2/den and drains
  lagging so the in-order engine queues never head-block.

Self-contained: hardcodes shapes; imports concourse from /opt/trn_rl_repo.
"""

import sys

if "/opt/trn_rl_repo" not in sys.path:
    sys.path.insert(0, "/opt/trn_rl_repo")

import numpy as np
import ml_dtypes

import concourse.bass as bass
import concourse.mybir as mybir
import concourse.tile as tile
from concourse import bacc

F32 = mybir.dt.float32
BF16 = mybir.dt.bfloat16
I16 = mybir.dt.int16

N_CORES = 8
B, NH, S, D = 4, 16, 2048, 64
H = (B * NH) // N_CORES  # heads per core = 8
P = 128
W = 256                  # query window
NWIN = S // W            # 8 windows per head
NWQ = NWIN // 4          # 2 window-quads per head
SCALE = 1.0 / 8.0

# Two-point Schraudolph exp on DVE: exp(x) ~ S(x+h) + S(x-h) where S is the
# classic bitcast-exp (tensor_scalar ->int16 whose int16 bits are the bf16
# pattern of exp/2cosh(h)); averaging two quarter-period-shifted sawtooths
# cuts the interpolation ripple to ~0.8% rms. The 1/(2cosh h) factor is
# folded into the bias constants.
LOG2E = 1.4426950408889634
INT_SHIFT = 78.0
_H_SH = INT_SHIFT / (256.0 * LOG2E)  # ~0.2112
_BS = 16256.0 - 128.0 * float(np.log2(2.0 * np.cosh(_H_SH))) - 7.0
EXP_A = 128.0 * LOG2E * SCALE
EXP_B_P = _BS + 128.0 * LOG2E * _H_SH
# macro i -> DVE iff i % ASSIGN_MOD in DVE_SLOTS (~2/7 of key-tiles approx);
# whole-macro grains amortize each engine's fixed per-instruction cost over
# FD=1024 (a column-split pays DVE fixed costs on ~250-element slices, which
# measured 3.7 ns/elem vs ScalarE's 1.17). First/last macros stay on ScalarE
# so the pipeline head/tail never waits on a long (~2.3us) DVE macro.
ASSIGN_MOD = 7
DVE_SLOTS = (2, 5)
MM2_LAG = 3
DRAIN_LAG = 5


def emit_core_program(ctx, nc, tc, qT_h, kT_h, vP_h, mk_h, accO_h, denO_h, T_K):
    """qT: [H,128,S] bf16; kT: [H,128,S_K] bf16; vP: [H,128,T_K,64] bf16;
    mk: [128,T_K,32] bf16 key-validity (col 0; cols 1-31 zero so den
    matmuls write all partitions, keeping the drained tile fully init); accO: [H,2,128,2,256] f32 raw numerator
    accumulators; denO: [H,2,128,256] f32 (rows 0/32/64/96 are the 4 windows'
    denominators). Host divides/transposes."""
    pool = lambda *a, **kw: ctx.enter_context(tc.tile_pool(*a, **kw))
    singles = pool(name="singles", bufs=1)
    ld = pool(name="ld", bufs=2)
    pT_pool = pool(name="pT", bufs=4)
    pab_pool = pool(name="pab", bufs=2)
    sbst_pool = pool(name="sbst", bufs=2)
    st_pool = pool(name="st", bufs=2, space="PSUM")    # [128,4,256] f32 = 2 banks
    acc_pool = pool(name="acc", bufs=2, space="PSUM")  # [128,2,256] f32 = 1 bank
    den_pool = pool(name="den", bufs=2, space="PSUM")  # [128,256] f32 = 1 bank

    # Pull the ~2.7us exp table load under the first input DMAs.
    scr = singles.tile([P, 8], F32, name="scr")
    nc.vector.memset(scr, 0.0)
    nc.scalar.activation(
        out=scr, in_=scr, func=mybir.ActivationFunctionType.Exp, scale=1.0
    )
    mk_sb = singles.tile([P, T_K, 32], BF16, name="mk_sb")

    def emit_head_load(h):
        q_sb = ld.tile([P, S], BF16, tag="q_sb", name=f"q_sb_{h}")
        k_sb = ld.tile([P, T_K * P], BF16, tag="k_sb", name=f"k_sb_{h}")
        v_sb = ld.tile([P, T_K, D], BF16, tag="v_sb", name=f"v_sb_{h}")
        if h == 0:
            # warmup: split first loads so macro 0 (k-tiles 0-1, windows 0-3)
            # can start ASAP; spread issue over two HWDGE queues (the
            # Activation queue is idle before the first exp)
            nc.scalar.dma_start(out=k_sb, in_=kT_h[h])
            nc.sync.dma_start(out=q_sb[:, 0 : 4 * W], in_=qT_h[h][:, 0 : 4 * W])
            nc.sync.dma_start(out=q_sb[:, 4 * W :], in_=qT_h[h][:, 4 * W :])
            nc.scalar.dma_start(out=v_sb, in_=vP_h[h])
            nc.scalar.dma_start(out=mk_sb, in_=mk_h)
        else:
            nc.sync.dma_start(out=q_sb, in_=qT_h[h])
            nc.sync.dma_start(out=k_sb, in_=kT_h[h])
            nc.sync.dma_start(out=v_sb, in_=vP_h[h])
        return q_sb, k_sb, v_sb

    # macros: (h, wq, j) — one k-tile of one window-quad
    macros = [
        (h, wq, j)
        for h in range(H)
        for wq in range(NWQ)
        for j in range(T_K)
    ]
    heads = {0: emit_head_load(0)}
    accden = {}
    pTs = {}
    pending_drain = []

    def emit_front(i):
        h, wq, j = macros[i]
        if wq == 0 and j == 0 and h > 1:
            del heads[h - 2]
        q_sb, k_sb, _ = heads[h]
        if j == 0:
            acc = acc_pool.tile([P, 2, W], F32, tag="acc", name=f"acc_{h}_{wq}")
            den = den_pool.tile([P, W], F32, tag="den", name=f"den_{h}_{wq}")
            accden[(h, wq)] = (acc, den)
        st = st_pool.tile([P, 4, W], F32, tag="st", name=f"st_{i}")
        # Concurrent row-group matmuls draining into the SAME psum bank are a
        # fatal HW collision: map window ww -> slot so row-group-0 windows
        # (ww even) fill bank 0 (slots 0,1) and row-group-1 windows fill bank
        # 1 (slots 2,3), while emission alternates groups for concurrency.
        for ww in range(4):
            c = ww & 1
            lo = D * c
            slot = 2 * c + (ww >> 1)
            wcol = (wq * 4 + ww) * W
            nc.tensor.matmul(
                st[:, slot, :],
                lhsT=k_sb[lo : lo + D, j * P : (j + 1) * P],
                rhs=q_sb[lo : lo + D, wcol : wcol + W],
                start=True,
                stop=True,
            )
        pT = pT_pool.tile([P, 4, W], BF16, tag="pT", name=f"pT_{i}")
        if i % ASSIGN_MOD in DVE_SLOTS and 2 <= i < len(macros) - 4:
            pa = pab_pool.tile([P, 4, W], I16, tag="pa", name=f"pa_{i}")
            pb = pab_pool.tile([P, 4, W], I16, tag="pb", name=f"pb_{i}")
            nc.vector.tensor_scalar(
                pa, st, EXP_A, EXP_B_P,
                mybir.AluOpType.mult, mybir.AluOpType.add,
            )
            nc.vector.tensor_scalar(
                pb, pa, -INT_SHIFT, 0.0,
                mybir.AluOpType.add, mybir.AluOpType.add,
            )
            nc.vector.tensor_add(pT, pa.bitcast(BF16), pb.bitcast(BF16))
        else:
            nc.scalar.activation(
                out=pT,
                in_=st,
                func=mybir.ActivationFunctionType.Exp,
                scale=SCALE,
            )
        pTs[i] = pT
        # prefetch the next head early in this head's macro stream
        if i % (NWQ * T_K) == min(1, NWQ * T_K - 1) and h + 1 < H:
            heads[h + 1] = emit_head_load(h + 1)

    def emit_back(i):
        h, wq, j = macros[i]
        v_sb = heads[h][2]
        pT = pTs.pop(i)
        acc, den = accden[(h, wq)]
        first = j == 0
        last = j == T_K - 1
        # acc packs 4 windows into ONE psum bank (2 partition-halves x 2
        # column-halves); den packs 4 M=1 rows into one bank. The has_written
        # clear on start=True covers the zero_region (the 2KB bank) of the
        # WRITTEN partitions only, so each partition-half carries start on
        # its first matmul (r==0) — the r==1 window's bytes land on cleared
        # bits and overwrite — and stop on its last. The sim's zero-region
        # group tracker aliases these sub-bank groups -> skip_group_check.
        slot = lambda ww: 2 * (ww & 1) + (ww >> 1)
        for r in range(2):
            nc.tensor.matmul(
                acc[0:D, r, :],
                lhsT=v_sb[:, j, :],
                rhs=pT[:, slot(2 * r), :],
                start=first and r == 0,
                stop=last and r == 1,
                skip_group_check=True,
            )
            nc.tensor.matmul(
                acc[D : 2 * D, r, :],
                lhsT=v_sb[:, j, :],
                rhs=pT[:, slot(2 * r + 1), :],
                start=first and r == 0,
                stop=last and r == 1,
                skip_group_check=True,
            )
        for g in range(4):
            nc.tensor.matmul(
                den[32 * g : 32 * g + 32, :],
                lhsT=mk_sb[:, j, :],
                rhs=pT[:, slot(g), :],
                start=first,
                stop=last,
                tile_position=(0, 32 * g),
                skip_group_check=True,
            )
        if last:
            del accden[(h, wq)]
            pending_drain.append((i + DRAIN_LAG, (h, wq, acc, den)))

    def emit_drain(ep):
        h, wq, acc, den = ep
        accs = sbst_pool.tile([P, 2, W], F32, tag="accs", name=f"accs_{h}_{wq}")
        nc.vector.tensor_copy(accs, acc)
        dens = sbst_pool.tile([P, W], F32, tag="dens", name=f"dens_{h}_{wq}")
        nc.vector.tensor_copy(dens, den)
        nc.sync.dma_start(out=accO_h[h][wq], in_=accs)
        nc.sync.dma_start(out=denO_h[h][wq], in_=dens)

    n = len(macros)
    for i in range(n):
        emit_front(i)
        if i >= MM2_LAG:
            emit_back(i - MM2_LAG)
        while pending_drain and pending_drain[0][0] <= i - MM2_LAG:
            emit_drain(pending_drain.pop(0)[1])
    for i in range(n - MM2_LAG, n):
        emit_back(i)
        while pending_drain and pending_drain[0][0] <= i:
            emit_drain(pending_drain.pop(0)[1])
    for _, ep in pending_drain:
        emit_drain(ep)


def build_nc(T_K):
    S_K = T_K * P
    nc = bacc.Bacc("TRN2", target_bir_lowering=False, debug=False, num_devices=N_CORES)
    qT = nc.declare_dram_parameter("qT", [H, P, S], BF16, isOutput=False)
    kT = nc.declare_dram_parameter("kT", [H, P, S_K], BF16, isOutput=False)
    vP = nc.declare_dram_parameter("vP", [H, P, T_K, D], BF16, isOutput=False)
    mk = nc.declare_dram_parameter("mk", [P, T_K, 32], BF16, isOutput=False)
    accO = nc.declare_dram_parameter("acc", [H, NWQ, P, 2, W], F32, isOutput=True)
    denO = nc.declare_dram_parameter("den", [H, NWQ, P, W], F32, isOutput=True)
    from contextlib import ExitStack

    with tile.TileContext(nc) as tc, ExitStack() as ctx:
        emit_core_program(
            ctx, nc, tc, qT.ap(), kT.ap(), vP.ap(), mk.ap(), accO.ap(), denO.ap(), T_K
        )
    nc.compile()
    return nc


_NC_CACHE = {}


def get_nc(T_K):
    if T_K not in _NC_CACHE:
        _NC_CACHE[T_K] = build_nc(T_K)
    return _NC_CACHE[T_K]


def make_in_maps(q, k, v, mask):
    """Host prep: compaction, transposes, duplication, swizzle, bf16 cast."""
    bf16 = ml_dtypes.bfloat16
    qf = np.asarray(q, dtype=np.float32).reshape(B * NH, S, D)
    kf = np.asarray(k, dtype=np.float32).reshape(B * NH, S, D)
    vf = np.asarray(v, dtype=np.float32).reshape(B * NH, S, D)
    mf = np.asarray(mask, dtype=np.int32).reshape(B, S)
    idxs = [np.nonzero(mf[b] == 0)[0] for b in range(B)]
    max_nu = max(len(ix) for ix in idxs)
    T_K = max(1, -(-max_nu // P))  # ceil
    S_K = T_K * P

    in_maps = []
    for c in range(N_CORES):
        lo = c * H
        b = lo // NH
        ix = idxs[b]
        nu = len(ix)
        qT = np.empty((H, P, S), dtype=bf16)
        kT = np.zeros((H, P, S_K), dtype=bf16)
        vP = np.zeros((H, P, T_K, D), dtype=bf16)
        mk = np.zeros((P, T_K, 32), dtype=bf16)
        kidx = np.arange(T_K)[None, :] * P + np.arange(P)[:, None]
        mk[:, :, 0] = (kidx < nu).astype(bf16)  # valid iff t*128+p < nu
        for hh in range(H):
            qt = np.ascontiguousarray(qf[lo + hh].T).astype(bf16)  # [D, S]
            qT[hh, 0:D] = qt
            qT[hh, D : 2 * D] = qt
            kg = kf[lo + hh][ix]  # [nu, D]
            kt = kg.T.astype(bf16)  # [D, nu]
            kT[hh, 0:D, 0:nu] = kt
            kT[hh, D : 2 * D, 0:nu] = kt
            vg = np.zeros((S_K, D), dtype=np.float32)
            vg[0:nu] = vf[lo + hh][ix]
            vP[hh] = vg.reshape(T_K, P, D).transpose(1, 0, 2).astype(bf16)
        in_maps.append({"qT": qT, "kT": kT, "vP": vP, "mk": mk})
    return in_maps, T_K


def kernel(q, k, v, mask):
    from concourse.bass_utils import run_bass_kernel_spmd

    in_maps, T_K = make_in_maps(q, k, v, mask)
    nc = get_nc(T_K)
    try:
        res = run_bass_kernel_spmd(nc, in_maps, list(range(N_CORES))).results
    except Exception:
        # transient INTERNAL error after a fresh NEFF compile; retry clears it
        res = run_bass_kernel_spmd(nc, in_maps, list(range(N_CORES))).results
    out = np.empty((B * NH, S, D), dtype=np.float32)
    for c in range(N_CORES):
        acc = res[c]["acc"]  # [H, NWQ, 128, 2, 256]
        den = res[c]["den"]  # [H, NWQ, 128, 256]
        for hh in range(H):
            for wq in range(NWQ):
                a = acc[hh, wq]
                d = den[hh, wq]
                for ww in range(4):
                    w = wq * 4 + ww
                    nums = a[(ww & 1) * D : (ww & 1) * D + D, ww >> 1, :]  # [64,256]
                    dd = d[32 * ww, :]  # [256]
                    out[c * H + hh, w * W : (w + 1) * W, :] = (nums / dd).T
    return out.reshape(B, NH, S, D)


if __name__ == "__main__":
    nc = build_nc(9)
    print("built ok")


# revision 29
# speedup vs baseline: 1.2200x; 1.0230x over previous
"""Trainium2 Bass kernel v5 for nn_BaseAttention (B=4, H=16, S=2048, D=64, key-mask).

Strategy (8 NeuronCores, 8 heads/core, all heads of a core share one batch's
mask):
  Host-side prep (exactness-preserving):
    - Key compaction: only unmasked keys are shipped (the reference's -10000
      additive mask zeroes masked keys exactly in fp32); K,V rows gathered by
      mask==0, zero-padded to S_K = T_K*128.
    - Q^T/K^T transposed on host and duplicated onto partitions 64..127
      (bf16) so mm1 runs two 64-row matmuls concurrently in the two PE row
      halves. V is swizzled to [128, T_K, 64] (no ones column — the softmax
      denominator has its own matmuls now).
  Device per macro (head, window-quad wq of 4x256 queries, k-tile j):
    - mm1: 4 matmuls (4 windows, row-tiled concurrent pairs) write S^T fp32
      into one PSUM tile [128, 4, 256] (2 banks; TRN2 matmul cannot emit
      16-bit PSUM).
    - exp: ONE ScalarE activation over the whole macro tile (FD=4*WSC,
      amortizing the ~260ns fixed ACT cost) for the first WSC columns of
      each window; the DVE covers the last DC=256-WSC columns with a
      two-point Schraudolph (one-point measured 2.2e-2 end-to-end: too big).
    - mm2: V-stationary, col-tiled: windows (2r, 2r+1) run CONCURRENTLY in
      PE column groups 0-1 / 2-3 (M=64 each), accumulating out^T into a
      single PSUM bank [128, 2, 256] (4 windows packed by partition-half x
      column-half).
    - den: 4 concurrent col-tiled M=1 matmuls (masked-ones weights kill the
      zero-padded keys) accumulate the 4 windows' softmax denominators into
      rows 0/32/64/96 of one PSUM bank.
    - epilogue: DVE drains acc+den to SBUF, DMA raw accumulators out; host
      divides and transposes.
  PSUM budget: st 2 banks x2 bufs + acc 1 bank x2 + den 1 bank x2 = 8.
  Emission is a flat software pipeline over macros with mm2/den and drains
  lagging so the in-order engine queues never head-block.

Self-contained: hardcodes shapes; imports concourse from /opt/trn_rl_repo.
"""

import sys

if "/opt/trn_rl_repo" not in sys.path:
    sys.path.insert(0, "/opt/trn_rl_repo")

import numpy as np
import ml_dtypes

import concourse.bass as bass
import concourse.mybir as mybir
import concourse.tile as tile
from concourse import bacc

F32 = mybir.dt.float32
BF16 = mybir.dt.bfloat16
I16 = mybir.dt.int16

N_CORES = 8
B, NH, S, D = 4, 16, 2048, 64
H = (B * NH) // N_CORES  # heads per core = 8
P = 128
W = 256                  # query window
NWIN = S // W            # 8 windows per head
NWQ = NWIN // 4          # 2 window-quads per head
SCALE = 1.0 / 8.0

# Two-point Schraudolph exp on DVE: exp(x) ~ S(x+h) + S(x-h) where S is the
# classic bitcast-exp (tensor_scalar ->int16 whose int16 bits are the bf16
# pattern of exp/2cosh(h)); averaging two quarter-period-shifted sawtooths
# cuts the interpolation ripple to ~0.8% rms. The 1/(2cosh h) factor is
# folded into the bias constants.
LOG2E = 1.4426950408889634
INT_SHIFT = 78.0
_H_SH = INT_SHIFT / (256.0 * LOG2E)  # ~0.2112
_BS = 16256.0 - 128.0 * float(np.log2(2.0 * np.cosh(_H_SH))) - 7.0
EXP_A = 128.0 * LOG2E * SCALE
EXP_B_P = _BS + 128.0 * LOG2E * _H_SH
# macro i -> DVE iff i % ASSIGN_MOD in DVE_SLOTS (~2/7 of key-tiles approx);
# whole-macro grains amortize each engine's fixed per-instruction cost over
# FD=1024 (a column-split pays DVE fixed costs on ~250-element slices, which
# measured 3.7 ns/elem vs ScalarE's 1.17). First/last macros stay on ScalarE
# so the pipeline head/tail never waits on a long (~2.3us) DVE macro.
ASSIGN_MOD = 7
DVE_SLOTS = (2, 5)
MM2_LAG = 3
DRAIN_LAG = 2


def emit_core_program(ctx, nc, tc, qT_h, kT_h, vP_h, mk_h, accO_h, denO_h, T_K):
    """qT: [H,128,S] bf16; kT: [H,128,S_K] bf16; vP: [H,128,T_K,64] bf16;
    mk: [128,T_K,32] bf16 key-validity (col 0; cols 1-31 zero so den
    matmuls write all partitions, keeping the drained tile fully init); accO: [H,2,128,2,256] f32 raw numerator
    accumulators; denO: [H,2,128,256] f32 (rows 0/32/64/96 are the 4 windows'
    denominators). Host divides/transposes."""
    pool = lambda *a, **kw: ctx.enter_context(tc.tile_pool(*a, **kw))
    singles = pool(name="singles", bufs=1)
    ld = pool(name="ld", bufs=2)
    pT_pool = pool(name="pT", bufs=4)
    pab_pool = pool(name="pab", bufs=2)
    sbst_pool = pool(name="sbst", bufs=2)
    st_pool = pool(name="st", bufs=2, space="PSUM")    # [128,4,256] f32 = 2 banks
    acc_pool = pool(name="acc", bufs=2, space="PSUM")  # [128,2,256] f32 = 1 bank
    den_pool = pool(name="den", bufs=2, space="PSUM")  # [128,256] f32 = 1 bank

    # Pull the ~2.7us exp table load under the first input DMAs.
    scr = singles.tile([P, 8], F32, name="scr")
    nc.vector.memset(scr, 0.0)
    nc.scalar.activation(
        out=scr, in_=scr, func=mybir.ActivationFunctionType.Exp, scale=1.0
    )
    mk_sb = singles.tile([P, T_K, 32], BF16, name="mk_sb")

    def emit_head_load(h):
        q_sb = ld.tile([P, S], BF16, tag="q_sb", name=f"q_sb_{h}")
        k_sb = ld.tile([P, T_K * P], BF16, tag="k_sb", name=f"k_sb_{h}")
        v_sb = ld.tile([P, T_K, D], BF16, tag="v_sb", name=f"v_sb_{h}")
        if h == 0:
            # warmup: split first loads so macro 0 (k-tiles 0-1, windows 0-3)
            # can start ASAP; spread issue over two HWDGE queues (the
            # Activation queue is idle before the first exp)
            nc.scalar.dma_start(out=k_sb[:, 0:P], in_=kT_h[h][:, 0:P])
            nc.sync.dma_start(out=q_sb[:, 0 : 4 * W], in_=qT_h[h][:, 0 : 4 * W])
            if T_K > 1:
                nc.sync.dma_start(out=k_sb[:, P:], in_=kT_h[h][:, P:])
            nc.sync.dma_start(out=q_sb[:, 4 * W :], in_=qT_h[h][:, 4 * W :])
            nc.scalar.dma_start(out=v_sb, in_=vP_h[h])
            nc.scalar.dma_start(out=mk_sb, in_=mk_h)
        else:
            nc.sync.dma_start(out=q_sb, in_=qT_h[h])
            nc.sync.dma_start(out=k_sb, in_=kT_h[h])
            nc.sync.dma_start(out=v_sb, in_=vP_h[h])
        return q_sb, k_sb, v_sb

    # macros: (h, wq, j) — one k-tile of one window-quad
    macros = [
        (h, wq, j)
        for h in range(H)
        for wq in range(NWQ)
        for j in range(T_K)
    ]
    heads = {0: emit_head_load(0)}
    accden = {}
    pTs = {}
    pending_drain = []

    def emit_front(i):
        h, wq, j = macros[i]
        if wq == 0 and j == 0 and h > 1:
            del heads[h - 2]
        q_sb, k_sb, _ = heads[h]
        if j == 0:
            acc = acc_pool.tile([P, 2, W], F32, tag="acc", name=f"acc_{h}_{wq}")
            den = den_pool.tile([P, W], F32, tag="den", name=f"den_{h}_{wq}")
            accden[(h, wq)] = (acc, den)
        st = st_pool.tile([P, 4, W], F32, tag="st", name=f"st_{i}")
        # Concurrent row-group matmuls draining into the SAME psum bank are a
        # fatal HW collision: map window ww -> slot so row-group-0 windows
        # (ww even) fill bank 0 (slots 0,1) and row-group-1 windows fill bank
        # 1 (slots 2,3), while emission alternates groups for concurrency.
        for ww in range(4):
            c = ww & 1
            lo = D * c
            slot = 2 * c + (ww >> 1)
            wcol = (wq * 4 + ww) * W
            nc.tensor.matmul(
                st[:, slot, :],
                lhsT=k_sb[lo : lo + D, j * P : (j + 1) * P],
                rhs=q_sb[lo : lo + D, wcol : wcol + W],
                start=True,
                stop=True,
            )
        pT = pT_pool.tile([P, 4, W], BF16, tag="pT", name=f"pT_{i}")
        if i % ASSIGN_MOD in DVE_SLOTS and 2 <= i < len(macros) - 4:
            pa = pab_pool.tile([P, 4, W], I16, tag="pa", name=f"pa_{i}")
            pb = pab_pool.tile([P, 4, W], I16, tag="pb", name=f"pb_{i}")
            nc.vector.tensor_scalar(
                pa, st, EXP_A, EXP_B_P,
                mybir.AluOpType.mult, mybir.AluOpType.add,
            )
            nc.vector.tensor_scalar(
                pb, pa, -INT_SHIFT, 0.0,
                mybir.AluOpType.add, mybir.AluOpType.add,
            )
            nc.vector.tensor_add(pT, pa.bitcast(BF16), pb.bitcast(BF16))
        else:
            nc.scalar.activation(
                out=pT,
                in_=st,
                func=mybir.ActivationFunctionType.Exp,
                scale=SCALE,
            )
        pTs[i] = pT
        # prefetch the next head early in this head's macro stream
        if i % (NWQ * T_K) == min(1, NWQ * T_K - 1) and h + 1 < H:
            heads[h + 1] = emit_head_load(h + 1)

    def emit_back(i):
        h, wq, j = macros[i]
        v_sb = heads[h][2]
        pT = pTs.pop(i)
        acc, den = accden[(h, wq)]
        first = j == 0
        last = j == T_K - 1
        # acc packs 4 windows into ONE psum bank (2 partition-halves x 2
        # column-halves); den packs 4 M=1 rows into one bank. The has_written
        # clear on start=True covers the zero_region (the 2KB bank) of the
        # WRITTEN partitions only, so each partition-half carries start on
        # its first matmul (r==0) — the r==1 window's bytes land on cleared
        # bits and overwrite — and stop on its last. The sim's zero-region
        # group tracker aliases these sub-bank groups -> skip_group_check.
        slot = lambda ww: 2 * (ww & 1) + (ww >> 1)
        for r in range(2):
            nc.tensor.matmul(
                acc[0:D, r, :],
                lhsT=v_sb[:, j, :],
                rhs=pT[:, slot(2 * r), :],
                start=first and r == 0,
                stop=last and r == 1,
                skip_group_check=True,
            )
            nc.tensor.matmul(
                acc[D : 2 * D, r, :],
                lhsT=v_sb[:, j, :],
                rhs=pT[:, slot(2 * r + 1), :],
                start=first and r == 0,
                stop=last and r == 1,
                skip_group_check=True,
            )
        for g in range(4):
            nc.tensor.matmul(
                den[32 * g : 32 * g + 32, :],
                lhsT=mk_sb[:, j, :],
                rhs=pT[:, slot(g), :],
                start=first,
                stop=last,
                tile_position=(0, 32 * g),
                skip_group_check=True,
            )
        if last:
            del accden[(h, wq)]
            pending_drain.append((i + DRAIN_LAG, (h, wq, acc, den)))

    def emit_drain(ep):
        h, wq, acc, den = ep
        accs = sbst_pool.tile([P, 2, W], F32, tag="accs", name=f"accs_{h}_{wq}")
        nc.vector.tensor_copy(accs, acc)
        dens = sbst_pool.tile([P, W], F32, tag="dens", name=f"dens_{h}_{wq}")
        nc.scalar.copy(dens, den)
        nc.sync.dma_start(out=accO_h[h][wq], in_=accs)
        nc.sync.dma_start(out=denO_h[h][wq], in_=dens)

    n = len(macros)
    for i in range(n):
        emit_front(i)
        if i >= MM2_LAG:
            emit_back(i - MM2_LAG)
        while pending_drain and pending_drain[0][0] <= i - MM2_LAG:
            emit_drain(pending_drain.pop(0)[1])
    for i in range(n - MM2_LAG, n):
        emit_back(i)
        while pending_drain and pending_drain[0][0] <= i:
            emit_drain(pending_drain.pop(0)[1])
    for _, ep in pending_drain:
        emit_drain(ep)


def build_nc(T_K):
    S_K = T_K * P
    nc = bacc.Bacc("TRN2", target_bir_lowering=False, debug=False, num_devices=N_CORES)
    qT = nc.declare_dram_parameter("qT", [H, P, S], BF16, isOutput=False)
    kT = nc.declare_dram_parameter("kT", [H, P, S_K], BF16, isOutput=False)
    vP = nc.declare_dram_parameter("vP", [H, P, T_K, D], BF16, isOutput=False)
    mk = nc.declare_dram_parameter("mk", [P, T_K, 32], BF16, isOutput=False)
    accO = nc.declare_dram_parameter("acc", [H, NWQ, P, 2, W], F32, isOutput=True)
    denO = nc.declare_dram_parameter("den", [H, NWQ, P, W], F32, isOutput=True)
    from contextlib import ExitStack

    with tile.TileContext(nc) as tc, ExitStack() as ctx:
        emit_core_program(
            ctx, nc, tc, qT.ap(), kT.ap(), vP.ap(), mk.ap(), accO.ap(), denO.ap(), T_K
        )
    nc.compile()
    return nc


_NC_CACHE = {}


def get_nc(T_K):
    if T_K not in _NC_CACHE:
        _NC_CACHE[T_K] = build_nc(T_K)
    return _NC_CACHE[T_K]


def make_in_maps(q, k, v, mask):
    """Host prep: compaction, transposes, duplication, swizzle, bf16 cast."""
    bf16 = ml_dtypes.bfloat16
    qf = np.asarray(q, dtype=np.float32).reshape(B * NH, S, D)
    kf = np.asarray(k, dtype=np.float32).reshape(B * NH, S, D)
    vf = np.asarray(v, dtype=np.float32).reshape(B * NH, S, D)
    mf = np.asarray(mask, dtype=np.int32).reshape(B, S)
    idxs = [np.nonzero(mf[b] == 0)[0] for b in range(B)]
    max_nu = max(len(ix) for ix in idxs)
    T_K = max(1, -(-max_nu // P))  # ceil
    S_K = T_K * P

    in_maps = []
    for c in range(N_CORES):
        lo = c * H
        b = lo // NH
        ix = idxs[b]
        nu = len(ix)
        qT = np.empty((H, P, S), dtype=bf16)
        kT = np.zeros((H, P, S_K), dtype=bf16)
        vP = np.zeros((H, P, T_K, D), dtype=bf16)
        mk = np.zeros((P, T_K, 32), dtype=bf16)
        kidx = np.arange(T_K)[None, :] * P + np.arange(P)[:, None]
        mk[:, :, 0] = (kidx < nu).astype(bf16)  # valid iff t*128+p < nu
        for hh in range(H):
            qt = np.ascontiguousarray(qf[lo + hh].T).astype(bf16)  # [D, S]
            qT[hh, 0:D] = qt
            qT[hh, D : 2 * D] = qt
            kg = kf[lo + hh][ix]  # [nu, D]
            kt = kg.T.astype(bf16)  # [D, nu]
            kT[hh, 0:D, 0:nu] = kt
            kT[hh, D : 2 * D, 0:nu] = kt
            vg = np.zeros((S_K, D), dtype=np.float32)
            vg[0:nu] = vf[lo + hh][ix]
            vP[hh] = vg.reshape(T_K, P, D).transpose(1, 0, 2).astype(bf16)
        in_maps.append({"qT": qT, "kT": kT, "vP": vP, "mk": mk})
    return in_maps, T_K


def kernel(q, k, v, mask):
    from concourse.bass_utils import run_bass_kernel_spmd

    in_maps, T_K = make_in_maps(q, k, v, mask)
    nc = get_nc(T_K)
    try:
        res = run_bass_kernel_spmd(nc, in_maps, list(range(N_CORES))).results
    except Exception:
        # transient INTERNAL error after a fresh NEFF compile; retry clears it
        res = run_bass_kernel_spmd(nc, in_maps, list(range(N_CORES))).results
    out = np.empty((B * NH, S, D), dtype=np.float32)
    for c in range(N_CORES):
        acc = res[c]["acc"]  # [H, NWQ, 128, 2, 256]
        den = res[c]["den"]  # [H, NWQ, 128, 256]
        for hh in range(H):
            for wq in range(NWQ):
                a = acc[hh, wq]
                d = den[hh, wq]
                for ww in range(4):
                    w = wq * 4 + ww
                    nums = a[(ww & 1) * D : (ww & 1) * D + D, ww >> 1, :]  # [64,256]
                    dd = d[32 * ww, :]  # [256]
                    out[c * H + hh, w * W : (w + 1) * W, :] = (nums / dd).T
    return out.reshape(B, NH, S, D)


if __name__ == "__main__":
    nc = build_nc(9)
    print("built ok")
